# revision 2
# baseline (speedup 1.0000x reference)
"""BQuantConv1d Trainium2 kernel.

Math: the reference's 256-entry LUT gather per (token, group, out-feature) is
algebraically out = X @ W' + bias with a single dense weight matrix
    W'[i, f] = sum_k scale[k, f] * sgn(bit_{7-(i%8)}(binary[0, k, i//8, f]))
(the per-plane scale is a per-output-column factor, so the 8 sign-matmuls of
the bit planes collapse into one matmul once scale is folded into the weights
on the host — the same host-side combine the plane-sharded variant did after
the fact, just moved before the matmul).

Device program per core (output-feature sharding, 96 features per core):
  - one packed input DMA [128, 2112] bf16: x^T in 6 K-tiles of 128 input
    features ([128, 6*256]) followed by this core's W' column slice in the
    matching K-tile layout ([128, 6*96]),
  - 6 PSUM-accumulated matmuls (stationary = W' tile [128, 96], streaming =
    x^T tile [128, 256]) producing out[f, b] = [96, 256] f32,
  - PSUM -> SBUF copy on DVE, output DMA [96, 256] f32.
X is replicated across cores; W'/out are column-sharded. The host transposes/
concatenates the 8 slices and adds bias.

Timing structure: the For_i hardware loop carries an all-engine barrier at
the back edge, so the body is emitted UNROLL times per loop iteration
(n_iter total body executions overall) — the barrier amortizes and bodies
double-buffer against each other through the tile pools.
"""

import numpy as np
import ml_dtypes

B = 256          # flattened tokens 4*64
NX = 768         # input features
NF = 768         # output features
NCORES = 8
BITS = 8         # kept for compatibility (== NCORES)
FS = NF // NCORES  # 96 output features per core
KT = 6           # contraction tiles of 128
XW = KT * B      # 1536 columns of x^T
COLS = XW + KT * FS  # 2112 packed input columns
UNROLL = 8

_CACHE = {}


def _emit_body(nc, tc, bass, mybir, pools, inp_d, out_d, mode="full"):
    fp32 = mybir.dt.float32
    bf16 = mybir.dt.bfloat16
    const, opool, psum = pools

    if mode == "empty":
        zz = const.tile([128, 1], fp32, tag="zz", name="zz")
        nc.gpsimd.memset(zz[:], 0.0)
        return

    inp = const.tile([128, COLS], bf16, tag="inp", name="inp")
    nc.sync.dma_start(inp[:], inp_d.ap())
    if mode == "dma":
        return

    out_sb = opool.tile([128, B], fp32, tag="out", name="out_sb")
    if mode == "nomm":
        nc.gpsimd.memset(out_sb[0:FS, :], 0.0)
    else:
        pm = psum.tile([128, B], fp32, tag="pm", name="pm")
        for t in range(KT):
            nc.tensor.matmul(
                pm[0:FS, :],
                inp[:, XW + t * FS : XW + (t + 1) * FS],
                inp[:, t * B : (t + 1) * B],
                start=(t == 0),
                stop=(t == KT - 1),
            )
        nc.vector.tensor_scalar(
            out_sb[0:FS, :], pm[0:FS, :], 0.0, None, mybir.AluOpType.add
        )
    nc.scalar.dma_start(out_d.ap(), out_sb[0:FS, :])


def _declare_io(nc, mybir):
    fp32 = mybir.dt.float32
    bf16 = mybir.dt.bfloat16
    # packed: cols [0, XW) = x^T K-tiles, cols [XW, COLS) = W' column slice
    inp_d = nc.dram_tensor("inp", [128, COLS], bf16, kind="ExternalInput")
    # out[f_local, b] — final output slice (pre-bias), f-major
    out_d = nc.dram_tensor("out", [FS, B], fp32, kind="ExternalOutput")
    return inp_d, out_d


def _build_program(n_iter=1, mode="full", unroll=UNROLL):
    import concourse.bass as bass
    import concourse.tile as tile
    from concourse import bacc, mybir

    nc = bacc.Bacc("TRN2", target_bir_lowering=False, debug=False)
    io = _declare_io(nc, mybir)

    with tile.TileContext(nc) as tc:
        with (
            tc.tile_pool(name="const", bufs=3) as const,
            tc.tile_pool(name="opool", bufs=2) as opool,
            tc.tile_pool(name="psum", bufs=2, space=bass.MemorySpace.PSUM) as psum,
        ):
            pools = (const, opool, psum)
            if n_iter == 1:
                _emit_body(nc, tc, bass, mybir, pools, *io, mode=mode)
            else:
                assert n_iter % unroll == 0, (n_iter, unroll)
                with tc.For_i(0, n_iter // unroll, 1):
                    for _ in range(unroll):
                        _emit_body(nc, tc, bass, mybir, pools, *io, mode=mode)

    nc.compile()
    return nc


def _prep_inputs(x, binary, scale):
    xf = np.asarray(x, dtype=np.float32).reshape(B, NX)
    # xT[p, t*B + b] = xf[b, t*128 + p]
    xT = np.ascontiguousarray(
        xf.T.reshape(KT, 128, B).transpose(1, 0, 2)
    ).reshape(128, XW).astype(ml_dtypes.bfloat16)

    bins = np.asarray(binary)[0].astype(np.uint8)        # [8, 96, 768]
    bits = np.unpackbits(bins[:, :, :, None], axis=3)    # [..., p] = bit (7-p)
    sgn = bits.astype(np.float32) * 2.0 - 1.0            # [8k, 96m, 768f, 8p]
    sc = np.asarray(scale, dtype=np.float32)[0]          # [8, 768]
    W = np.einsum("kmfp,kf->mpf", sgn, sc).reshape(NX, NF)
    Wr = W.reshape(KT, 128, NCORES, FS)                  # [t, p, j, f]

    in_maps = []
    for j in range(NCORES):
        wj = np.ascontiguousarray(Wr[:, :, j, :].transpose(1, 0, 2)).reshape(
            128, KT * FS
        ).astype(ml_dtypes.bfloat16)
        inp = np.ascontiguousarray(np.concatenate([xT, wj], axis=1))
        in_maps.append({"inp": inp})
    return in_maps


def kernel(x, scale, binary, bias, _trace=False):
    from concourse.bass_utils import run_bass_kernel_spmd

    if "nc" not in _CACHE:
        _CACHE["nc"] = _build_program()
    nc = _CACHE["nc"]

    in_maps = _prep_inputs(x, binary, scale)
    res = run_bass_kernel_spmd(nc, in_maps, core_ids=list(range(NCORES)), trace=_trace)
    _CACHE["last_result"] = res

    outT = np.concatenate(
        [np.asarray(res.results[j]["out"]) for j in range(NCORES)], axis=0
    )  # [768, 256]
    out = outT.T + np.asarray(bias, dtype=np.float32)[None, :]
    return out.reshape(4, 64, NF).astype(np.float32)
